# revision 2
# baseline (speedup 1.0000x reference)
"""RNN-T JointNet kernel for Trainium2, 8 NeuronCores.

Reference computation (B=4, T=256, U=64, D=640, H=640, V=1024):
    enc  = enc_out @ W_enc + b_enc          (B,T,H)
    pred = pred_out @ W_pred + b_pred       (B,U,H)
    joint = tanh(enc[:,:,None,:] + pred[:,None,:,:])
    logits = joint @ W_fc + b_fc            (B,T,U,V)
    out = log_softmax(logits, -1)

Sharding: 1024 (b,t) rows split into 8 chunks of 128; core i gets batch
b=i//2, t-rows (i%2)*128..+128 and computes its full (128,U,V) slab.

Strategy (engine-balanced, fp8 DoubleRow):
  - Host passes encT/predT pre-transposed bf16, W_enc/W_pred bf16, and
    W_fc*16 packed as fp8e4m3 k-tiles [w0,w1,w2,w3,ZERO,w4].
  - PE: projections in bf16; joint matmul all-DoubleRow (0.5 cyc/row):
    jwr k-pairs (0,1),(2,3),(3,4) vs wfc pairs (w0,w1),(w2,w3),(ZERO,w4)
    so the odd k-tile count needs no zero-padding of jwr.
  - Pool: broadcast-add epT[k]+ppbT[k][:,u] (bf16) and out = lg - logS.
  - ACT: tanh (fp8 out) and exp; one act table (exp_and_others) serves
    tanh+exp for the whole program -> a single table load.
  - DVE: lg = psum*(1/16)+b_fc (frees PSUM fast), S-sums via free
    accum_out on bf16 copies, logS = bit-trick fast-log + one Newton
    step (tiny exp on ACT, same table).
  - Software-pipelined emission (program order == Tile scheduler
    priority): tanh quads lead matmul/exp quads by 2, accum/logS/sub/DMA
    tails trail by 1.
  - Output staged bf16, upcast to f32 on host.
"""

import numpy as np
import ml_dtypes
from contextlib import ExitStack

import concourse.bass as bass
import concourse.bacc as bacc
import concourse.tile as tile
from concourse import mybir
from concourse.bass_utils import run_bass_kernel_spmd

F32 = mybir.dt.float32
BF16 = mybir.dt.bfloat16
FP8 = mybir.dt.float8e4
DR = mybir.MatmulPerfMode.DoubleRow
Tanh = mybir.ActivationFunctionType.Tanh
Exp = mybir.ActivationFunctionType.Exp
Ln = mybir.ActivationFunctionType.Ln

B, T, U = 4, 256, 64
D, H, V = 640, 640, 1024
NCORES = 8
TC = (B * T) // NCORES        # 128 t-rows per core
KT = H // 128                 # 5 contraction tiles
KT6 = KT + 1                  # +1 pad/bias tile for DoubleRow pairs
UB = 8                        # u-block size
NB = U // UB                  # 8 blocks
QU = 4                        # u's per exp quad
SCALE = 16.0
INV_SCALE = 1.0 / SCALE


def _build_module():
    nc = bacc.Bacc()
    encT = nc.declare_dram_parameter("encT", [D, TC], BF16, isOutput=False)
    predT = nc.declare_dram_parameter("predT", [D, U], BF16, isOutput=False)
    w_enc = nc.declare_dram_parameter("w_enc", [D, H], BF16, isOutput=False)
    w_pred = nc.declare_dram_parameter("w_pred", [D, H], BF16, isOutput=False)
    wfc8 = nc.declare_dram_parameter("wfc8", [KT6, 128, V], FP8, isOutput=False)
    bc = nc.declare_dram_parameter("bc", [H], F32, isOutput=False)
    bfc2 = nc.declare_dram_parameter("bfc2", [2 * V], BF16, isOutput=False)
    out = nc.declare_dram_parameter("out", [TC, U, V], BF16, isOutput=True)

    with ExitStack() as ctx:
        tc_ = ctx.enter_context(tile.TileContext(nc))
        _body(ctx, tc_, encT, predT, w_enc, w_pred, wfc8, bc, bfc2, out)
    nc.compile()
    return nc


def _body(ctx, tc, encT, predT, w_enc, w_pred, wfc8, bc, bfc2, out):
    nc = tc.nc

    singles = ctx.enter_context(tc.tile_pool(name="singles", bufs=1))

    # ---- persistent tiles ----
    wfc_sb = singles.tile([128, KT6 * V], FP8)
    nc.gpsimd.dma_start(out=wfc_sb[:].rearrange("p (k v) -> p k v", k=KT6),
                        in_=wfc8[:, :, :].rearrange("k p v -> p k v"))
    bc_sb = singles.tile([128, KT], F32)
    nc.sync.dma_start(out=bc_sb, in_=bc[:].rearrange("(k p) -> p k", p=128))
    epT = singles.tile([128, KT * TC], BF16)       # [h_p, (k,t)]
    ppbT = singles.tile([128, KT * U], F32)        # [h_p, (k,u)]
    S_sb = singles.tile([128, U], F32)
    logS_sb = singles.tile([128, U], F32)

    # ---- prologue: load + project (scoped pools so PSUM/SBUF free) ----
    with tc.tile_pool(name="pro", bufs=1) as pro, \
         tc.tile_pool(name="pro_ps", bufs=4, space="PSUM") as pro_ps:
        encT_sb = pro.tile([128, KT * TC], BF16)   # [d_p, (k,t)]
        nc.sync.dma_start(out=encT_sb[:].rearrange("p (k t) -> p k t", k=KT),
                          in_=encT[:, :].rearrange("(k p) t -> p k t", p=128))
        predT_sb = pro.tile([128, KT * U], BF16)
        nc.sync.dma_start(out=predT_sb[:].rearrange("p (k u) -> p k u", k=KT),
                          in_=predT[:, :].rearrange("(k p) u -> p k u", p=128))
        wenc_sb = pro.tile([128, KT * H], BF16)    # [d_p, (k,h)]
        nc.scalar.dma_start(out=wenc_sb[:].rearrange("p (k h) -> p k h", k=KT),
                            in_=w_enc[:, :].rearrange("(k p) h -> p k h", p=128))
        wpred_sb = pro.tile([128, KT * H], BF16)
        nc.gpsimd.dma_start(out=wpred_sb[:].rearrange("p (k h) -> p k h", k=KT),
                            in_=w_pred[:, :].rearrange("(k p) h -> p k h", p=128))

        for m in range(KT):
            ps = pro_ps.tile([128, TC], F32, tag="proj_e")
            for k in range(KT):
                nc.tensor.matmul(ps, wenc_sb[:, k * H + m * 128:k * H + (m + 1) * 128],
                                 encT_sb[:, k * TC:(k + 1) * TC],
                                 start=(k == 0), stop=(k == KT - 1))
            nc.vector.tensor_copy(epT[:, m * TC:(m + 1) * TC], ps)
        for m in range(KT):
            ps = pro_ps.tile([128, U], F32, tag="proj_p")
            for k in range(KT):
                nc.tensor.matmul(ps, wpred_sb[:, k * H + m * 128:k * H + (m + 1) * 128],
                                 predT_sb[:, k * U:(k + 1) * U],
                                 start=(k == 0), stop=(k == KT - 1))
            # fold b_enc+b_pred while copying out of PSUM
            nc.vector.tensor_scalar_add(ppbT[:, m * U:(m + 1) * U], ps,
                                        bc_sb[:, m:m + 1])

    # ---- bias replicated across partitions (ones-column matmul trick) ----
    bias_rep = singles.tile([128, 2 * V], F32)
    with tc.tile_pool(name="bia", bufs=1) as bia, \
         tc.tile_pool(name="bia_ps", bufs=1, space="PSUM") as bia_ps:
        ones1 = bia.tile([1, 128], BF16)
        nc.vector.memset(ones1, 1.0)
        bfc2_sb = bia.tile([1, 2 * V], BF16)
        nc.sync.dma_start(out=bfc2_sb,
                          in_=bfc2[:].rearrange("(o v) -> o v", o=1))
        psb = bia_ps.tile([128, 2 * V], F32)
        for s in range(4):
            nc.tensor.matmul(psb[:, s * 512:(s + 1) * 512], ones1,
                             bfc2_sb[:, s * 512:(s + 1) * 512],
                             start=True, stop=True)
        nc.vector.tensor_copy(bias_rep, psb)

    # ---- pools for the main pipeline ----
    jwpool = ctx.enter_context(tc.tile_pool(name="jw", bufs=3))
    jrpool = ctx.enter_context(tc.tile_pool(name="jwr", bufs=4))
    psum = ctx.enter_context(tc.tile_pool(name="psum", bufs=4, space="PSUM"))
    lgpool = ctx.enter_context(tc.tile_pool(name="lg", bufs=5))
    expool = ctx.enter_context(tc.tile_pool(name="ex", bufs=3))
    scpool = ctx.enter_context(tc.tile_pool(name="scr", bufs=4))
    obpool = ctx.enter_context(tc.tile_pool(name="ob", bufs=8))
    smpool = ctx.enter_context(tc.tile_pool(name="sm", bufs=3))

    # ---- software-pipelined main loop ----
    # Program order == Tile scheduler priority, so emission order shapes
    # the schedule: tanh quads lead the matmul/exp quads by 2, tails
    # (accums/logS/subs/DMA) trail by 1.
    # jw/jwr layout per block: [128h_p, (k,u,t)] — the k-pair stride stays
    # 1024 elems so DoubleRow pair APs can't be coalesced away.
    # All-DoubleRow matmuls: jwr pairs (0,1),(2,3),(3,4) vs wfc pairs
    # (w0,w1),(w2,w3),(ZERO,w4) — k3 contributes 0 in the last pair.
    LOG2 = 0.6931471805599453
    FLC1 = LOG2 / (1 << 23)
    FLC2 = (127.0 + 0.0430357) * LOG2
    wfc4 = wfc_sb[:].rearrange("p (k v) -> p k v", k=KT6)
    QPB = UB // QU
    NQ = NB * QPB                     # total quads
    ex_tiles = [None] * NQ
    lg_tiles = [None] * NQ
    jw_blocks = [None] * NB
    jwr_blocks = [None] * NB

    def emit_tanh_quad(TQ):
        b, qq_ = TQ // QPB, TQ % QPB
        if qq_ == 0:
            jw_blocks[b] = jwpool.tile([128, KT * UB * TC], BF16, tag="jw",
                                       name=f"jw{b}")
            jwr_blocks[b] = jrpool.tile([128, KT * UB * TC], FP8, tag="jwr",
                                        name=f"jwr{b}")
        jw, jwr = jw_blocks[b], jwr_blocks[b]
        for ul in range(qq_ * QU, (qq_ + 1) * QU):
            u = b * UB + ul
            for k in range(KT):
                nc.gpsimd.tensor_scalar_add(
                    jw[:, (k * UB + ul) * TC:(k * UB + ul + 1) * TC],
                    epT[:, k * TC:(k + 1) * TC],
                    ppbT[:, k * U + u:k * U + u + 1])
        jw4 = jw[:].rearrange("p (k u t) -> p k u t", k=KT, u=UB)
        jr4 = jwr[:].rearrange("p (k u t) -> p k u t", k=KT, u=UB)
        nc.scalar.activation(jr4[:, :, qq_ * QU:(qq_ + 1) * QU, :],
                             jw4[:, :, qq_ * QU:(qq_ + 1) * QU, :], Tanh)

    def emit_quad(Q):
        b, q = Q // QPB, Q % QPB
        jwr4 = jwr_blocks[b][:].rearrange("p (k u t) -> p k u t", k=KT, u=UB)
        lg = lgpool.tile([128, QU * V], BF16, tag="lg")
        for j in range(QU):
            ul = q * QU + j
            ps = psum.tile([128, V], F32, tag="logits")
            for vh in range(2):
                for kp, js in enumerate((0, 2, 3)):
                    nc.tensor.matmul(
                        ps[:, vh * 512:(vh + 1) * 512],
                        jwr4[:, js:js + 2, ul:ul + 1, :],
                        wfc4[:, 2 * kp:2 * kp + 2, vh * 512:(vh + 1) * 512],
                        start=(kp == 0), stop=(kp == 2), perf_mode=DR)
            # lg = psum*(1/SCALE) + b_fc  (frees psum; bias folded in free)
            nc.vector.scalar_tensor_tensor(
                lg[:, j * V:(j + 1) * V], ps, INV_SCALE,
                bias_rep[:, 0:V], mybir.AluOpType.mult, mybir.AluOpType.add)
        ex = expool.tile([128, QU * V], BF16, tag="ex")
        nc.scalar.activation(ex, lg, Exp)
        ex_tiles[Q] = ex
        lg_tiles[Q] = lg

    def emit_tail(Q):
        # accums (DVE), logS via fast-log + Newton (Pool + one tiny ACT
        # exp), subs (Pool) and DMA for quad Q
        b, q = Q // QPB, Q % QPB
        ex = ex_tiles[Q]
        u0 = b * UB + q * QU
        for j in range(QU):
            u = u0 + j
            scr = scpool.tile([128, V], BF16, tag="scr")
            nc.vector.tensor_scalar(scr, ex[:, j * V:(j + 1) * V], 1.0,
                                    0.0, mybir.AluOpType.mult,
                                    mybir.AluOpType.add,
                                    accum_out=S_sb[:, u:u + 1])
        sblk = S_sb[:, u0:u0 + QU]
        sf = smpool.tile([128, QU], F32, tag="sf")
        nc.vector.tensor_copy(sf, sblk.bitcast(mybir.dt.uint32))
        ls0 = smpool.tile([128, QU], F32, tag="ls0")
        nc.vector.tensor_scalar(ls0, sf, FLC1, FLC2, mybir.AluOpType.mult,
                                mybir.AluOpType.subtract)
        r = smpool.tile([128, QU], F32, tag="r")
        nc.scalar.activation(r, ls0, Exp)
        rinv = smpool.tile([128, QU], F32, tag="rinv")
        nc.vector.reciprocal_approx_fast(rinv, r)
        qt = smpool.tile([128, QU], F32, tag="qt")
        nc.vector.tensor_tensor(qt, sblk, rinv, mybir.AluOpType.mult)
        nc.vector.scalar_tensor_tensor(
            logS_sb[:, u0:u0 + QU], qt, -1.0, ls0,
            mybir.AluOpType.add, mybir.AluOpType.add)
        lg = lg_tiles[Q]
        for jj in range(QU // 2):
            ob = obpool.tile([128, 2 * V], BF16, tag="ob")
            for h in range(2):
                j = jj * 2 + h
                eng = nc.vector if (Q >= NQ - 2 and h == 1) else nc.gpsimd
                eng.tensor_scalar_sub(
                    ob[:, h * V:(h + 1) * V], lg[:, j * V:(j + 1) * V],
                    logS_sb[:, u0 + j:u0 + j + 1])
            nc.sync.dma_start(out=out[:, u0 + jj * 2:u0 + jj * 2 + 2, :],
                              in_=ob)

    LEAD = 2
    for step in range(NQ + LEAD + 1):
        if step < NQ:
            emit_tanh_quad(step)
        if LEAD <= step < NQ + LEAD:
            emit_quad(step - LEAD)
        if step > LEAD:
            emit_tail(step - LEAD - 1)
    emit_tail(NQ - 1)


_NC_CACHE = None


def _get_module():
    global _NC_CACHE
    if _NC_CACHE is None:
        _NC_CACHE = _build_module()
    return _NC_CACHE


def kernel(enc_out, pred_out, W_enc, b_enc, W_pred, b_pred, W_fc, b_fc):
    nc = _get_module()
    enc_out = np.asarray(enc_out, dtype=np.float32)
    pred_out = np.asarray(pred_out, dtype=np.float32)
    W_fc = np.asarray(W_fc, dtype=np.float32)
    b_fc = np.asarray(b_fc, dtype=np.float32)

    wfc8 = np.zeros((KT6, 128, V), dtype=ml_dtypes.float8_e4m3)
    for k in range(4):
        wfc8[k] = (W_fc[k * 128:(k + 1) * 128, :] * SCALE).astype(
            ml_dtypes.float8_e4m3)
    wfc8[5] = (W_fc[4 * 128:5 * 128, :] * SCALE).astype(
        ml_dtypes.float8_e4m3)

    shared = {
        "bfc2": np.tile(b_fc, 2).astype(ml_dtypes.bfloat16),
        "w_enc": np.ascontiguousarray(W_enc, dtype=np.float32).astype(
            ml_dtypes.bfloat16),
        "w_pred": np.ascontiguousarray(W_pred, dtype=np.float32).astype(
            ml_dtypes.bfloat16),
        "wfc8": wfc8,
        "bc": np.ascontiguousarray(
            np.asarray(b_enc, np.float32) + np.asarray(b_pred, np.float32)),
    }
    in_maps = []
    for i in range(NCORES):
        b = i // (T // TC)
        t0 = (i % (T // TC)) * TC
        in_maps.append({
            "encT": np.ascontiguousarray(
                enc_out[b, t0:t0 + TC, :].T).astype(ml_dtypes.bfloat16),
            "predT": np.ascontiguousarray(
                pred_out[b].T).astype(ml_dtypes.bfloat16),
            **shared,
        })
    res = run_bass_kernel_spmd(nc, in_maps, core_ids=list(range(NCORES)))
    full = np.empty((B, T, U, V), dtype=np.float32)
    for i in range(NCORES):
        b = i // (T // TC)
        t0 = (i % (T // TC)) * TC
        full[b, t0:t0 + TC] = res.results[i]["out"].astype(np.float32)
    return full


# revision 3
# speedup vs baseline: 1.0108x; 1.0108x over previous
"""RNN-T JointNet kernel for Trainium2, 8 NeuronCores.

Reference computation (B=4, T=256, U=64, D=640, H=640, V=1024):
    enc  = enc_out @ W_enc + b_enc          (B,T,H)
    pred = pred_out @ W_pred + b_pred       (B,U,H)
    joint = tanh(enc[:,:,None,:] + pred[:,None,:,:])
    logits = joint @ W_fc + b_fc            (B,T,U,V)
    out = log_softmax(logits, -1)

Sharding: 1024 (b,t) rows split into 8 chunks of 128; core i gets batch
b=i//2, t-rows (i%2)*128..+128 and computes its full (128,U,V) slab.

Strategy (engine-balanced, fp8 DoubleRow):
  - Host passes encT/predT pre-transposed bf16, W_enc/W_pred bf16, and
    W_fc*16 packed as fp8e4m3 k-tiles [w0,w1,w2,w3,ZERO,w4].
  - PE: projections in bf16; joint matmul all-DoubleRow (0.5 cyc/row):
    jwr k-pairs (0,1),(2,3),(3,4) vs wfc pairs (w0,w1),(w2,w3),(ZERO,w4)
    so the odd k-tile count needs no zero-padding of jwr.
  - Pool: broadcast-add epT[k]+ppbT[k][:,u] (bf16) and out = lg - logS.
  - ACT: tanh (fp8 out) and exp; one act table (exp_and_others) serves
    tanh+exp for the whole program -> a single table load.
  - DVE: lg = psum*(1/16)+b_fc (frees PSUM fast), S-sums via free
    accum_out on bf16 copies, logS = bit-trick fast-log + one Newton
    step (tiny exp on ACT, same table).
  - Software-pipelined emission (program order == Tile scheduler
    priority): tanh quads lead matmul/exp quads by 3, tails trail by 1;
    prologue weight DMAs and the final quads' subs/DMAs are spread
    across the SP/ACT/Pool queues to shorten fill and drain.
  - Output staged bf16, upcast to f32 on host.
"""

import numpy as np
import ml_dtypes
from contextlib import ExitStack

import concourse.bass as bass
import concourse.bacc as bacc
import concourse.tile as tile
from concourse import mybir
from concourse.bass_utils import run_bass_kernel_spmd

F32 = mybir.dt.float32
BF16 = mybir.dt.bfloat16
FP8 = mybir.dt.float8e4
DR = mybir.MatmulPerfMode.DoubleRow
Tanh = mybir.ActivationFunctionType.Tanh
Exp = mybir.ActivationFunctionType.Exp
Ln = mybir.ActivationFunctionType.Ln

B, T, U = 4, 256, 64
D, H, V = 640, 640, 1024
NCORES = 8
TC = (B * T) // NCORES        # 128 t-rows per core
KT = H // 128                 # 5 contraction tiles
KT6 = KT + 1                  # +1 pad/bias tile for DoubleRow pairs
UB = 8                        # u-block size
NB = U // UB                  # 8 blocks
QU = 4                        # u's per exp quad
SCALE = 16.0
INV_SCALE = 1.0 / SCALE


def _build_module():
    nc = bacc.Bacc()
    encT = nc.declare_dram_parameter("encT", [D, TC], BF16, isOutput=False)
    predT = nc.declare_dram_parameter("predT", [D, U], BF16, isOutput=False)
    w_enc = nc.declare_dram_parameter("w_enc", [D, H], BF16, isOutput=False)
    w_pred = nc.declare_dram_parameter("w_pred", [D, H], BF16, isOutput=False)
    wfc8 = nc.declare_dram_parameter("wfc8", [KT6, 128, V], FP8, isOutput=False)
    bc = nc.declare_dram_parameter("bc", [H], F32, isOutput=False)
    bfc2 = nc.declare_dram_parameter("bfc2", [2 * V], BF16, isOutput=False)
    out = nc.declare_dram_parameter("out", [TC, U, V], BF16, isOutput=True)

    with ExitStack() as ctx:
        tc_ = ctx.enter_context(tile.TileContext(nc))
        _body(ctx, tc_, encT, predT, w_enc, w_pred, wfc8, bc, bfc2, out)
    nc.compile()
    return nc


def _body(ctx, tc, encT, predT, w_enc, w_pred, wfc8, bc, bfc2, out):
    nc = tc.nc

    singles = ctx.enter_context(tc.tile_pool(name="singles", bufs=1))

    # ---- persistent tiles ----
    wfc_sb = singles.tile([128, KT6 * V], FP8)
    nc.gpsimd.dma_start(out=wfc_sb[:].rearrange("p (k v) -> p k v", k=KT6),
                        in_=wfc8[:, :, :].rearrange("k p v -> p k v"))
    bc_sb = singles.tile([128, KT], F32)
    nc.sync.dma_start(out=bc_sb, in_=bc[:].rearrange("(k p) -> p k", p=128))
    epT = singles.tile([128, KT * TC], BF16)       # [h_p, (k,t)]
    ppbT = singles.tile([128, KT * U], F32)        # [h_p, (k,u)]
    S_sb = singles.tile([128, U], F32)
    logS_sb = singles.tile([128, U], F32)

    # ---- prologue: load + project (scoped pools so PSUM/SBUF free) ----
    with tc.tile_pool(name="pro", bufs=1) as pro, \
         tc.tile_pool(name="pro_ps", bufs=4, space="PSUM") as pro_ps:
        encT_sb = pro.tile([128, KT * TC], BF16)   # [d_p, (k,t)]
        nc.sync.dma_start(out=encT_sb[:].rearrange("p (k t) -> p k t", k=KT),
                          in_=encT[:, :].rearrange("(k p) t -> p k t", p=128))
        predT_sb = pro.tile([128, KT * U], BF16)
        nc.sync.dma_start(out=predT_sb[:].rearrange("p (k u) -> p k u", k=KT),
                          in_=predT[:, :].rearrange("(k p) u -> p k u", p=128))
        wenc_sb = pro.tile([128, KT * H], BF16)    # [d_p, (k,h)]
        nc.scalar.dma_start(out=wenc_sb[:].rearrange("p (k h) -> p k h", k=KT),
                            in_=w_enc[:, :].rearrange("(k p) h -> p k h", p=128))
        wpred_sb = pro.tile([128, KT * H], BF16)
        nc.gpsimd.dma_start(out=wpred_sb[:].rearrange("p (k h) -> p k h", k=KT),
                            in_=w_pred[:, :].rearrange("(k p) h -> p k h", p=128))

        for m in range(KT):
            ps = pro_ps.tile([128, TC], F32, tag="proj_e")
            for k in range(KT):
                nc.tensor.matmul(ps, wenc_sb[:, k * H + m * 128:k * H + (m + 1) * 128],
                                 encT_sb[:, k * TC:(k + 1) * TC],
                                 start=(k == 0), stop=(k == KT - 1))
            nc.scalar.copy(epT[:, m * TC:(m + 1) * TC], ps)
        for m in range(KT):
            ps = pro_ps.tile([128, U], F32, tag="proj_p")
            for k in range(KT):
                nc.tensor.matmul(ps, wpred_sb[:, k * H + m * 128:k * H + (m + 1) * 128],
                                 predT_sb[:, k * U:(k + 1) * U],
                                 start=(k == 0), stop=(k == KT - 1))
            # fold b_enc+b_pred while copying out of PSUM
            nc.scalar.add(ppbT[:, m * U:(m + 1) * U], ps,
                          bc_sb[:, m:m + 1])

    # ---- bias replicated across partitions (ones-column matmul trick) ----
    bias_rep = singles.tile([128, 2 * V], F32)
    with tc.tile_pool(name="bia", bufs=1) as bia, \
         tc.tile_pool(name="bia_ps", bufs=1, space="PSUM") as bia_ps:
        ones1 = bia.tile([1, 128], BF16)
        nc.vector.memset(ones1, 1.0)
        bfc2_sb = bia.tile([1, 2 * V], BF16)
        nc.sync.dma_start(out=bfc2_sb,
                          in_=bfc2[:].rearrange("(o v) -> o v", o=1))
        psb = bia_ps.tile([128, 2 * V], F32)
        for s in range(4):
            nc.tensor.matmul(psb[:, s * 512:(s + 1) * 512], ones1,
                             bfc2_sb[:, s * 512:(s + 1) * 512],
                             start=True, stop=True)
        nc.scalar.copy(bias_rep, psb)

    # ---- pools for the main pipeline ----
    jwpool = ctx.enter_context(tc.tile_pool(name="jw", bufs=3))
    jrpool = ctx.enter_context(tc.tile_pool(name="jwr", bufs=4))
    psum = ctx.enter_context(tc.tile_pool(name="psum", bufs=4, space="PSUM"))
    lgpool = ctx.enter_context(tc.tile_pool(name="lg", bufs=6))
    expool = ctx.enter_context(tc.tile_pool(name="ex", bufs=4))
    scpool = ctx.enter_context(tc.tile_pool(name="scr", bufs=4))
    obpool = ctx.enter_context(tc.tile_pool(name="ob", bufs=6))
    smpool = ctx.enter_context(tc.tile_pool(name="sm", bufs=3))

    # ---- software-pipelined main loop ----
    # Program order == Tile scheduler priority, so emission order shapes
    # the schedule: tanh quads lead the matmul/exp quads by 2, tails
    # (accums/logS/subs/DMA) trail by 1.
    # jw/jwr layout per block: [128h_p, (k,u,t)] — the k-pair stride stays
    # 1024 elems so DoubleRow pair APs can't be coalesced away.
    # All-DoubleRow matmuls: jwr pairs (0,1),(2,3),(3,4) vs wfc pairs
    # (w0,w1),(w2,w3),(ZERO,w4) — k3 contributes 0 in the last pair.
    LOG2 = 0.6931471805599453
    FLC1 = LOG2 / (1 << 23)
    FLC2 = (127.0 + 0.0430357) * LOG2
    wfc4 = wfc_sb[:].rearrange("p (k v) -> p k v", k=KT6)
    QPB = UB // QU
    NQ = NB * QPB                     # total quads
    ex_tiles = [None] * NQ
    lg_tiles = [None] * NQ
    jw_blocks = [None] * NB
    jwr_blocks = [None] * NB

    def emit_tanh_quad(TQ):
        b, qq_ = TQ // QPB, TQ % QPB
        if qq_ == 0:
            jw_blocks[b] = jwpool.tile([128, KT * UB * TC], BF16, tag="jw",
                                       name=f"jw{b}")
            jwr_blocks[b] = jrpool.tile([128, KT * UB * TC], FP8, tag="jwr",
                                        name=f"jwr{b}")
        jw, jwr = jw_blocks[b], jwr_blocks[b]
        for ul in range(qq_ * QU, (qq_ + 1) * QU):
            u = b * UB + ul
            for k in range(KT):
                nc.gpsimd.tensor_scalar_add(
                    jw[:, (k * UB + ul) * TC:(k * UB + ul + 1) * TC],
                    epT[:, k * TC:(k + 1) * TC],
                    ppbT[:, k * U + u:k * U + u + 1])
        jw4 = jw[:].rearrange("p (k u t) -> p k u t", k=KT, u=UB)
        jr4 = jwr[:].rearrange("p (k u t) -> p k u t", k=KT, u=UB)
        nc.scalar.activation(jr4[:, :, qq_ * QU:(qq_ + 1) * QU, :],
                             jw4[:, :, qq_ * QU:(qq_ + 1) * QU, :], Tanh)

    def emit_quad(Q):
        b, q = Q // QPB, Q % QPB
        jwr4 = jwr_blocks[b][:].rearrange("p (k u t) -> p k u t", k=KT, u=UB)
        lg = lgpool.tile([128, QU * V], BF16, tag="lg")
        for j in range(QU):
            ul = q * QU + j
            ps = psum.tile([128, V], F32, tag="logits")
            for vh in range(2):
                for kp, js in enumerate((0, 2, 3)):
                    nc.tensor.matmul(
                        ps[:, vh * 512:(vh + 1) * 512],
                        jwr4[:, js:js + 2, ul:ul + 1, :],
                        wfc4[:, 2 * kp:2 * kp + 2, vh * 512:(vh + 1) * 512],
                        start=(kp == 0), stop=(kp == 2), perf_mode=DR)
            # lg = psum*(1/SCALE) + b_fc  (frees psum; bias folded in free)
            nc.vector.scalar_tensor_tensor(
                lg[:, j * V:(j + 1) * V], ps, INV_SCALE,
                bias_rep[:, 0:V], mybir.AluOpType.mult, mybir.AluOpType.add)
        ex = expool.tile([128, QU * V], BF16, tag="ex")
        nc.scalar.activation(ex, lg, Exp)
        ex_tiles[Q] = ex
        lg_tiles[Q] = lg

    def emit_tail(Q):
        # accums (DVE), logS via fast-log + Newton (Pool + one tiny ACT
        # exp), subs (Pool) and DMA for quad Q
        b, q = Q // QPB, Q % QPB
        ex = ex_tiles[Q]
        u0 = b * UB + q * QU
        for j in range(QU):
            u = u0 + j
            scr = scpool.tile([128, V], BF16, tag="scr")
            nc.vector.tensor_scalar(scr, ex[:, j * V:(j + 1) * V], 1.0,
                                    0.0, mybir.AluOpType.mult,
                                    mybir.AluOpType.add,
                                    accum_out=S_sb[:, u:u + 1])
        sblk = S_sb[:, u0:u0 + QU]
        sf = smpool.tile([128, QU], F32, tag="sf")
        nc.vector.tensor_copy(sf, sblk.bitcast(mybir.dt.uint32))
        ls0 = smpool.tile([128, QU], F32, tag="ls0")
        nc.vector.tensor_scalar(ls0, sf, FLC1, FLC2, mybir.AluOpType.mult,
                                mybir.AluOpType.subtract)
        r = smpool.tile([128, QU], F32, tag="r")
        nc.scalar.activation(r, ls0, Exp)
        rinv = smpool.tile([128, QU], F32, tag="rinv")
        nc.vector.reciprocal_approx_fast(rinv, r)
        qt = smpool.tile([128, QU], F32, tag="qt")
        nc.vector.tensor_tensor(qt, sblk, rinv, mybir.AluOpType.mult)
        nc.vector.scalar_tensor_tensor(
            logS_sb[:, u0:u0 + QU], qt, -1.0, ls0,
            mybir.AluOpType.add, mybir.AluOpType.add)
        lg = lg_tiles[Q]
        for jj in range(QU // 2):
            ob = obpool.tile([128, 2 * V], BF16, tag="ob")
            for h in range(2):
                j = jj * 2 + h
                eng = nc.vector if (Q >= NQ - 2 and h == 1) else nc.gpsimd
                eng.tensor_scalar_sub(
                    ob[:, h * V:(h + 1) * V], lg[:, j * V:(j + 1) * V],
                    logS_sb[:, u0 + j:u0 + j + 1])
            dmae = (nc.sync, nc.scalar, nc.gpsimd)[jj % 3] \
                if Q >= NQ - 2 else nc.sync
            dmae.dma_start(out=out[:, u0 + jj * 2:u0 + jj * 2 + 2, :],
                           in_=ob)

    LEAD = 3
    for step in range(NQ + LEAD + 1):
        if step < NQ:
            emit_tanh_quad(step)
        if LEAD <= step < NQ + LEAD:
            emit_quad(step - LEAD)
        if step > LEAD:
            emit_tail(step - LEAD - 1)
    emit_tail(NQ - 1)


_NC_CACHE = None


def _get_module():
    global _NC_CACHE
    if _NC_CACHE is None:
        _NC_CACHE = _build_module()
    return _NC_CACHE


def kernel(enc_out, pred_out, W_enc, b_enc, W_pred, b_pred, W_fc, b_fc):
    nc = _get_module()
    enc_out = np.asarray(enc_out, dtype=np.float32)
    pred_out = np.asarray(pred_out, dtype=np.float32)
    W_fc = np.asarray(W_fc, dtype=np.float32)
    b_fc = np.asarray(b_fc, dtype=np.float32)

    wfc8 = np.zeros((KT6, 128, V), dtype=ml_dtypes.float8_e4m3)
    for k in range(4):
        wfc8[k] = (W_fc[k * 128:(k + 1) * 128, :] * SCALE).astype(
            ml_dtypes.float8_e4m3)
    wfc8[5] = (W_fc[4 * 128:5 * 128, :] * SCALE).astype(
        ml_dtypes.float8_e4m3)

    shared = {
        "bfc2": np.tile(b_fc, 2).astype(ml_dtypes.bfloat16),
        "w_enc": np.ascontiguousarray(W_enc, dtype=np.float32).astype(
            ml_dtypes.bfloat16),
        "w_pred": np.ascontiguousarray(W_pred, dtype=np.float32).astype(
            ml_dtypes.bfloat16),
        "wfc8": wfc8,
        "bc": np.ascontiguousarray(
            np.asarray(b_enc, np.float32) + np.asarray(b_pred, np.float32)),
    }
    in_maps = []
    for i in range(NCORES):
        b = i // (T // TC)
        t0 = (i % (T // TC)) * TC
        in_maps.append({
            "encT": np.ascontiguousarray(
                enc_out[b, t0:t0 + TC, :].T).astype(ml_dtypes.bfloat16),
            "predT": np.ascontiguousarray(
                pred_out[b].T).astype(ml_dtypes.bfloat16),
            **shared,
        })
    res = run_bass_kernel_spmd(nc, in_maps, core_ids=list(range(NCORES)))
    full = np.empty((B, T, U, V), dtype=np.float32)
    for i in range(NCORES):
        b = i // (T // TC)
        t0 = (i % (T // TC)) * TC
        full[b, t0:t0 + TC] = res.results[i]["out"].astype(np.float32)
    return full


# revision 4
# speedup vs baseline: 1.0127x; 1.0019x over previous
"""RNN-T JointNet kernel for Trainium2, 8 NeuronCores — v2.

Reference computation (B=4, T=256, U=64, D=640, H=640, V=1024):
    enc  = enc_out @ W_enc + b_enc          (B,T,H)
    pred = pred_out @ W_pred + b_pred       (B,U,H)
    joint = tanh(enc[:,:,None,:] + pred[:,None,:,:])
    logits = joint @ W_fc + b_fc            (B,T,U,V)
    out = log_softmax(logits, -1)

Sharding: 1024 (b,t) rows split into 8 chunks of 128; core i gets batch
b=i//2, t-rows (i%2)*128..+128 and computes its full (128,U,V) slab.

v2 strategy (engine-balanced, fp8 DoubleRow):
  - Host passes encT/predT pre-transposed bf16, W_enc/W_pred bf16,
    W_fc*16 packed as fp8e4m3 k-tiles with a 6th tile carrying b_fc*16
    in partition-row 0 (ones-row DoubleRow pair folds the bias in free).
  - PE: projections in bf16; joint matmul in fp8 DoubleRow (0.5 cyc/row,
    2 k-tiles per matmul): 3 DR matmuls per (u, v-half).
  - DVE: broadcast-add epT[k] + ppbT[k][:,u] in bf16 (4x mode), S-sum via
    free accum_out on a bf16 copy, final out = lg - logS in bf16 (4x).
  - ACT: phase 1 = all tanh (exp_and_others table), phase 2 = exp + ln
    (natural_log_exp_and_others table): only ~3 table loads.
  - Pool: lg = psum * (1/16) -> bf16 SBUF, freeing PSUM fast.
  - Output staged bf16, upcast to f32 on host.
"""

import numpy as np
import ml_dtypes
from contextlib import ExitStack

import concourse.bass as bass
import concourse.bacc as bacc
import concourse.tile as tile
from concourse import mybir
from concourse.bass_utils import run_bass_kernel_spmd

F32 = mybir.dt.float32
BF16 = mybir.dt.bfloat16
FP8 = mybir.dt.float8e4
DR = mybir.MatmulPerfMode.DoubleRow
Tanh = mybir.ActivationFunctionType.Tanh
Exp = mybir.ActivationFunctionType.Exp
Ln = mybir.ActivationFunctionType.Ln

B, T, U = 4, 256, 64
D, H, V = 640, 640, 1024
NCORES = 8
TC = (B * T) // NCORES        # 128 t-rows per core
KT = H // 128                 # 5 contraction tiles
KT6 = KT + 1                  # +1 pad/bias tile for DoubleRow pairs
UB = 8                        # u-block size
NB = U // UB                  # 8 blocks
QU = 4                        # u's per exp quad
SCALE = 16.0
INV_SCALE = 1.0 / SCALE


def _build_module():
    nc = bacc.Bacc()
    encT = nc.declare_dram_parameter("encT", [D, TC], BF16, isOutput=False)
    predT = nc.declare_dram_parameter("predT", [D, U], BF16, isOutput=False)
    w_enc = nc.declare_dram_parameter("w_enc", [D, H], BF16, isOutput=False)
    w_pred = nc.declare_dram_parameter("w_pred", [D, H], BF16, isOutput=False)
    wfc8 = nc.declare_dram_parameter("wfc8", [KT6, 128, V], FP8, isOutput=False)
    bc = nc.declare_dram_parameter("bc", [H], F32, isOutput=False)
    bfc2 = nc.declare_dram_parameter("bfc2", [2 * V], BF16, isOutput=False)
    out = nc.declare_dram_parameter("out", [TC, U, V], BF16, isOutput=True)

    with ExitStack() as ctx:
        tc_ = ctx.enter_context(tile.TileContext(nc))
        _body(ctx, tc_, encT, predT, w_enc, w_pred, wfc8, bc, bfc2, out)
    nc.compile()
    return nc


def _body(ctx, tc, encT, predT, w_enc, w_pred, wfc8, bc, bfc2, out):
    nc = tc.nc

    singles = ctx.enter_context(tc.tile_pool(name="singles", bufs=1))

    # ---- persistent tiles ----
    wfc_sb = singles.tile([128, KT6 * V], FP8)
    nc.gpsimd.dma_start(out=wfc_sb[:].rearrange("p (k v) -> p k v", k=KT6),
                        in_=wfc8[:, :, :].rearrange("k p v -> p k v"))
    bc_sb = singles.tile([128, KT], F32)
    nc.sync.dma_start(out=bc_sb, in_=bc[:].rearrange("(k p) -> p k", p=128))
    epT = singles.tile([128, KT * TC], BF16)       # [h_p, (k,t)]
    ppbT = singles.tile([128, KT * U], F32)        # [h_p, (k,u)]
    S_sb = singles.tile([128, U], F32)
    logS_sb = singles.tile([128, U], F32)

    # ---- prologue: load + project (scoped pools so PSUM/SBUF free) ----
    with tc.tile_pool(name="pro", bufs=1) as pro, \
         tc.tile_pool(name="pro_ps", bufs=4, space="PSUM") as pro_ps:
        encT_sb = pro.tile([128, KT * TC], BF16)   # [d_p, (k,t)]
        nc.sync.dma_start(out=encT_sb[:].rearrange("p (k t) -> p k t", k=KT),
                          in_=encT[:, :].rearrange("(k p) t -> p k t", p=128))
        predT_sb = pro.tile([128, KT * U], BF16)
        nc.sync.dma_start(out=predT_sb[:].rearrange("p (k u) -> p k u", k=KT),
                          in_=predT[:, :].rearrange("(k p) u -> p k u", p=128))
        wenc_sb = pro.tile([128, KT * H], BF16)    # [d_p, (k,h)]
        nc.scalar.dma_start(out=wenc_sb[:].rearrange("p (k h) -> p k h", k=KT),
                            in_=w_enc[:, :].rearrange("(k p) h -> p k h", p=128))
        wpred_sb = pro.tile([128, KT * H], BF16)
        nc.gpsimd.dma_start(out=wpred_sb[:].rearrange("p (k h) -> p k h", k=KT),
                            in_=w_pred[:, :].rearrange("(k p) h -> p k h", p=128))

        for m in range(KT):
            ps = pro_ps.tile([128, TC], F32, tag="proj_e")
            for k in range(KT):
                nc.tensor.matmul(ps, wenc_sb[:, k * H + m * 128:k * H + (m + 1) * 128],
                                 encT_sb[:, k * TC:(k + 1) * TC],
                                 start=(k == 0), stop=(k == KT - 1))
            nc.scalar.copy(epT[:, m * TC:(m + 1) * TC], ps)
        for m in range(KT):
            ps = pro_ps.tile([128, U], F32, tag="proj_p")
            for k in range(KT):
                nc.tensor.matmul(ps, wpred_sb[:, k * H + m * 128:k * H + (m + 1) * 128],
                                 predT_sb[:, k * U:(k + 1) * U],
                                 start=(k == 0), stop=(k == KT - 1))
            # fold b_enc+b_pred while copying out of PSUM
            nc.scalar.add(ppbT[:, m * U:(m + 1) * U], ps,
                          bc_sb[:, m:m + 1])

    # ---- bias replicated across partitions (ones-column matmul trick) ----
    bias_rep = singles.tile([128, 2 * V], F32)
    with tc.tile_pool(name="bia", bufs=1) as bia, \
         tc.tile_pool(name="bia_ps", bufs=1, space="PSUM") as bia_ps:
        ones1 = bia.tile([1, 128], BF16)
        nc.vector.memset(ones1, 1.0)
        bfc2_sb = bia.tile([1, 2 * V], BF16)
        nc.sync.dma_start(out=bfc2_sb,
                          in_=bfc2[:].rearrange("(o v) -> o v", o=1))
        psb = bia_ps.tile([128, 2 * V], F32)
        for s in range(4):
            nc.tensor.matmul(psb[:, s * 512:(s + 1) * 512], ones1,
                             bfc2_sb[:, s * 512:(s + 1) * 512],
                             start=True, stop=True)
        nc.scalar.copy(bias_rep, psb)

    # ---- pools for the main pipeline ----
    jwpool = ctx.enter_context(tc.tile_pool(name="jw", bufs=3))
    jrpool = ctx.enter_context(tc.tile_pool(name="jwr", bufs=4))
    psum = ctx.enter_context(tc.tile_pool(name="psum", bufs=2, space="PSUM"))
    lgpool = ctx.enter_context(tc.tile_pool(name="lg", bufs=6))
    expool = ctx.enter_context(tc.tile_pool(name="ex", bufs=4))
    scpool = ctx.enter_context(tc.tile_pool(name="scr", bufs=4))
    obpool = ctx.enter_context(tc.tile_pool(name="ob", bufs=6))
    smpool = ctx.enter_context(tc.tile_pool(name="sm", bufs=3))

    # ---- software-pipelined main loop ----
    # Program order == Tile scheduler priority, so emission order shapes
    # the schedule: tanh quads lead the matmul/exp quads by 2, tails
    # (accums/logS/subs/DMA) trail by 1.
    # jw/jwr layout per block: [128h_p, (k,u,t)] — the k-pair stride stays
    # 1024 elems so DoubleRow pair APs can't be coalesced away.
    # All-DoubleRow matmuls: jwr pairs (0,1),(2,3),(3,4) vs wfc pairs
    # (w0,w1),(w2,w3),(ZERO,w4) — k3 contributes 0 in the last pair.
    LOG2 = 0.6931471805599453
    FLC1 = LOG2 / (1 << 23)
    FLC2 = (127.0 + 0.0430357) * LOG2
    wfc4 = wfc_sb[:].rearrange("p (k v) -> p k v", k=KT6)
    QPB = UB // QU
    NQ = NB * QPB                     # total quads
    ex_tiles = [None] * NQ
    lg_tiles = [None] * NQ
    jw_blocks = [None] * NB
    jwr_blocks = [None] * NB

    def emit_tanh_quad(TQ):
        b, qq_ = TQ // QPB, TQ % QPB
        if qq_ == 0:
            jw_blocks[b] = jwpool.tile([128, KT * UB * TC], BF16, tag="jw",
                                       name=f"jw{b}")
            jwr_blocks[b] = jrpool.tile([128, KT * UB * TC], FP8, tag="jwr",
                                        name=f"jwr{b}")
        jw, jwr = jw_blocks[b], jwr_blocks[b]
        for ul in range(qq_ * QU, (qq_ + 1) * QU):
            u = b * UB + ul
            for k in range(KT):
                nc.gpsimd.tensor_scalar_add(
                    jw[:, (k * UB + ul) * TC:(k * UB + ul + 1) * TC],
                    epT[:, k * TC:(k + 1) * TC],
                    ppbT[:, k * U + u:k * U + u + 1])
        jw4 = jw[:].rearrange("p (k u t) -> p k u t", k=KT, u=UB)
        jr4 = jwr[:].rearrange("p (k u t) -> p k u t", k=KT, u=UB)
        nc.scalar.activation(jr4[:, :, qq_ * QU:(qq_ + 1) * QU, :],
                             jw4[:, :, qq_ * QU:(qq_ + 1) * QU, :], Tanh)

    def emit_quad(Q):
        b, q = Q // QPB, Q % QPB
        jwr4 = jwr_blocks[b][:].rearrange("p (k u t) -> p k u t", k=KT, u=UB)
        lg = lgpool.tile([128, QU * V], BF16, tag="lg")
        for half in range(QU // 2):
            ps = psum.tile([128, 2 * V], F32, tag="logits")
            for j2 in range(2):
                ul = q * QU + half * 2 + j2
                for vh in range(2):
                    for kp, js in enumerate((0, 2, 3)):
                        nc.tensor.matmul(
                            ps[:, (j2 * 2 + vh) * 512:(j2 * 2 + vh + 1) * 512],
                            jwr4[:, js:js + 2, ul:ul + 1, :],
                            wfc4[:, 2 * kp:2 * kp + 2, vh * 512:(vh + 1) * 512],
                            start=(kp == 0), stop=(kp == 2), perf_mode=DR)
            # lg = psum*(1/SCALE) + b_fc: one 2-u copy frees the psum tile
            nc.vector.scalar_tensor_tensor(
                lg[:, half * 2 * V:(half + 1) * 2 * V], ps, INV_SCALE,
                bias_rep, mybir.AluOpType.mult, mybir.AluOpType.add)
        ex = expool.tile([128, QU * V], BF16, tag="ex")
        nc.scalar.activation(ex, lg, Exp)
        ex_tiles[Q] = ex
        lg_tiles[Q] = lg

    def emit_tail(Q):
        # accums (DVE), logS via fast-log + Newton (Pool + one tiny ACT
        # exp), subs (Pool) and DMA for quad Q
        b, q = Q // QPB, Q % QPB
        ex = ex_tiles[Q]
        u0 = b * UB + q * QU
        for j in range(QU):
            u = u0 + j
            scr = scpool.tile([128, V], BF16, tag="scr")
            nc.vector.tensor_scalar(scr, ex[:, j * V:(j + 1) * V], 1.0,
                                    0.0, mybir.AluOpType.mult,
                                    mybir.AluOpType.add,
                                    accum_out=S_sb[:, u:u + 1])
        sblk = S_sb[:, u0:u0 + QU]
        sf = smpool.tile([128, QU], F32, tag="sf")
        nc.vector.tensor_copy(sf, sblk.bitcast(mybir.dt.uint32))
        ls0 = smpool.tile([128, QU], F32, tag="ls0")
        nc.vector.tensor_scalar(ls0, sf, FLC1, FLC2, mybir.AluOpType.mult,
                                mybir.AluOpType.subtract)
        r = smpool.tile([128, QU], F32, tag="r")
        nc.scalar.activation(r, ls0, Exp)
        rinv = smpool.tile([128, QU], F32, tag="rinv")
        nc.vector.reciprocal_approx_fast(rinv, r)
        qt = smpool.tile([128, QU], F32, tag="qt")
        nc.vector.tensor_tensor(qt, sblk, rinv, mybir.AluOpType.mult)
        nc.vector.scalar_tensor_tensor(
            logS_sb[:, u0:u0 + QU], qt, -1.0, ls0,
            mybir.AluOpType.add, mybir.AluOpType.add)
        lg = lg_tiles[Q]
        for jj in range(QU // 2):
            ob = obpool.tile([128, 2 * V], BF16, tag="ob")
            for h in range(2):
                j = jj * 2 + h
                eng = nc.vector if (Q >= NQ - 2 and h == 1) else nc.gpsimd
                eng.tensor_scalar_sub(
                    ob[:, h * V:(h + 1) * V], lg[:, j * V:(j + 1) * V],
                    logS_sb[:, u0 + j:u0 + j + 1])
            dmae = (nc.sync, nc.scalar, nc.gpsimd)[jj % 3] \
                if Q >= NQ - 2 else nc.sync
            dmae.dma_start(out=out[:, u0 + jj * 2:u0 + jj * 2 + 2, :],
                           in_=ob)

    LEAD = 3
    for step in range(NQ + LEAD + 1):
        if step < NQ:
            emit_tanh_quad(step)
        if LEAD <= step < NQ + LEAD:
            emit_quad(step - LEAD)
        if step > LEAD:
            emit_tail(step - LEAD - 1)
    emit_tail(NQ - 1)


_NC_CACHE = None


def _get_module():
    global _NC_CACHE
    if _NC_CACHE is None:
        _NC_CACHE = _build_module()
    return _NC_CACHE


def kernel(enc_out, pred_out, W_enc, b_enc, W_pred, b_pred, W_fc, b_fc):
    nc = _get_module()
    enc_out = np.asarray(enc_out, dtype=np.float32)
    pred_out = np.asarray(pred_out, dtype=np.float32)
    W_fc = np.asarray(W_fc, dtype=np.float32)
    b_fc = np.asarray(b_fc, dtype=np.float32)

    wfc8 = np.zeros((KT6, 128, V), dtype=ml_dtypes.float8_e4m3)
    for k in range(4):
        wfc8[k] = (W_fc[k * 128:(k + 1) * 128, :] * SCALE).astype(
            ml_dtypes.float8_e4m3)
    wfc8[5] = (W_fc[4 * 128:5 * 128, :] * SCALE).astype(
        ml_dtypes.float8_e4m3)

    shared = {
        "bfc2": np.tile(b_fc, 2).astype(ml_dtypes.bfloat16),
        "w_enc": np.ascontiguousarray(W_enc, dtype=np.float32).astype(
            ml_dtypes.bfloat16),
        "w_pred": np.ascontiguousarray(W_pred, dtype=np.float32).astype(
            ml_dtypes.bfloat16),
        "wfc8": wfc8,
        "bc": np.ascontiguousarray(
            np.asarray(b_enc, np.float32) + np.asarray(b_pred, np.float32)),
    }
    in_maps = []
    for i in range(NCORES):
        b = i // (T // TC)
        t0 = (i % (T // TC)) * TC
        in_maps.append({
            "encT": np.ascontiguousarray(
                enc_out[b, t0:t0 + TC, :].T).astype(ml_dtypes.bfloat16),
            "predT": np.ascontiguousarray(
                pred_out[b].T).astype(ml_dtypes.bfloat16),
            **shared,
        })
    res = run_bass_kernel_spmd(nc, in_maps, core_ids=list(range(NCORES)))
    full = np.empty((B, T, U, V), dtype=np.float32)
    for i in range(NCORES):
        b = i // (T // TC)
        t0 = (i % (T // TC)) * TC
        full[b, t0:t0 + TC] = res.results[i]["out"].astype(np.float32)
    return full
